# revision 1
# baseline (speedup 1.0000x reference)
"""Multi-head self-attention (b=2, n=2048, emb=1024, heads=16) on 8 trn2 cores.

Sharding: core c = (b, hg) with b = c // 4, hg = c % 4. Data parallel over
batch, tensor parallel over head-groups (4 heads / 256 emb-cols per core).
Each core computes Q/K/V projections for its heads, full attention for its
heads, and a partial output projection ctx_hg @ Wo[:, hg_slice].T of shape
[2048, 1024]. The host sums the 4 partials per batch (Megatron row-parallel
reduce done on host) and adds the rank-1 bias term bv @ Wo.T + bo.

Device layout notes:
- Host pre-transposes x -> xT [emb, n] and weight slices so every matmul
  contracts over the partition dim.
- Q^T, K^T are produced in [dq, n] layout (dq = head-major), V in natural
  [n, dv] layout augmented with a ones column per head -> the ctx matmul
  ctxT[65, nq] = V_aug^T @ E^T produces softmax row-sums in row 64 for free.
- exp(S^T) runs on ACT straight out of PSUM in up-to-1536-wide instructions;
  softmax normalization is deferred to the small ctx^T tile.
- All matmuls run in float16 (1 cyc/col on PE + fast weight load; 10-bit
  mantissa keeps the overall error ~7e-4 scale-relative, validated vs fp32).
- q/k biases are added on-device (free, fused into the PSUM->SBUF copy);
  v/o biases are exactly the rank-1 host-side term above.
"""

import os
import sys

for _p in ("/opt/trn_rl_repo", "/root/.axon_site/_ro/trn_rl_repo"):
    if os.path.isdir(_p) and _p not in sys.path:
        sys.path.append(_p)

import numpy as np

import concourse.bass as bass  # noqa: F401  (engine types pulled via nc)
import concourse.mybir as mybir
import concourse.tile as tile
from concourse import bacc
from concourse.bass_utils import run_bass_kernel_spmd

B, N, EMB, HEADS, HD = 2, 2048, 1024, 16, 64
N_CORES = 8
TP = 4                      # head-group shards per batch
DQ = EMB // TP              # 256 emb-cols (4 heads) per core
SCALE = HD ** -0.5          # 0.125

F32 = mybir.dt.float32
F16 = mybir.dt.float16
FP = mybir.ActivationFunctionType

NQ = 512                    # nq chunk for projections / out-proj (moving free dim)
NJ = N // NQ                # 4 nq chunks
NQA = 256                   # nq chunk for attention (so 6 nk-chunks fit one exp)
NJA = N // NQA              # 8 attention nq chunks
NKC = 128                   # nk chunk (ctx contraction)
NT = N // NKC               # 16 nk chunks
KC = EMB // 128             # 8 e chunks
# nk-chunk groups per exp instruction (4 x 256 -> 1024-wide exps).
# PSUM budget (8 banks): pp 2 + s0 2 + s1 2 + c0 1 + c1 1. pp is
# double-buffered so projection psum groups never head-of-line-block the
# in-order PE queue while attention S matmuls are behind them.
T_GROUPS_H = (
    [tuple(range(0, 4)), tuple(range(4, 8)), tuple(range(8, 12)),
     tuple(range(12, 16))],
    [tuple(range(0, 4)), tuple(range(4, 8)), tuple(range(8, 12)),
     tuple(range(12, 16))],
)


def build_program():
    """Build + compile the single SPMD program all 8 cores run."""
    nc = bacc.Bacc("TRN2", target_bir_lowering=False, debug=False,
                   num_devices=N_CORES)

    xT = nc.dram_tensor("xT", [EMB, N], F16, kind="ExternalInput").ap()
    wqT = nc.dram_tensor("wqT", [EMB, DQ], F16, kind="ExternalInput").ap()
    wkT = nc.dram_tensor("wkT", [EMB, DQ], F16, kind="ExternalInput").ap()
    wvT = nc.dram_tensor("wvT", [EMB, DQ], F16, kind="ExternalInput").ap()
    woT = nc.dram_tensor("woT", [DQ, EMB], F16, kind="ExternalInput").ap()
    bqd = nc.dram_tensor("bq_s", [DQ], F32, kind="ExternalInput").ap()
    bkd = nc.dram_tensor("bk_s", [DQ], F32, kind="ExternalInput").ap()
    out_part = nc.dram_tensor("out_part", [N, EMB], F32,
                              kind="ExternalOutput").ap()

    with tile.TileContext(nc) as tc:
        with (
            tc.tile_pool(name="const", bufs=1) as const,
            tc.tile_pool(name="xp", bufs=24) as xp,
            tc.tile_pool(name="persist", bufs=1) as persist,
            tc.tile_pool(name="epool", bufs=2) as epool,
            tc.tile_pool(name="npool", bufs=2) as npool,
            tc.tile_pool(name="opool", bufs=NT) as opool,
            # PSUM static budget (8 banks): pp 1 + s0 3 + s1 3 + c 1
            tc.tile_pool(name="ppool", bufs=2, space="PSUM") as ppool,
            tc.tile_pool(name="spool", bufs=1, space="PSUM") as spool,
            tc.tile_pool(name="cpool", bufs=1, space="PSUM") as cpool,
        ):
            # ---- constants ----
            # per-k-chunk weight DMAs: the first projection matmul only
            # depends on its own 64KB slice, not the whole weight
            wq_sb = const.tile([128, KC, DQ], F16, tag="wq")
            wk_sb = const.tile([128, KC, DQ], F16, tag="wk")
            wv_sb = const.tile([128, KC, DQ], F16, tag="wv")
            for k in range(KC):
                nc.sync.dma_start(out=wk_sb[:, k, :], in_=wkT.rearrange(
                    "(k p) d -> k p d", p=128)[k])
                nc.sync.dma_start(out=wv_sb[:, k, :], in_=wvT.rearrange(
                    "(k p) d -> k p d", p=128)[k])
                nc.sync.dma_start(out=wq_sb[:, k, :], in_=wqT.rearrange(
                    "(k p) d -> k p d", p=128)[k])
            # wo is needed only by the out-projection (~60us in) — its DMA
            # is deferred into the filler stream to keep startup queues clear
            wo_sb = const.tile([128, 2, EMB], F16, tag="wo")
            bq_sb = const.tile([128, 2], F32, tag="bq")
            nc.sync.dma_start(out=bq_sb, in_=bqd.rearrange("(m p) -> p m", p=128))
            bk_sb = const.tile([128, 2], F32, tag="bk")
            nc.sync.dma_start(out=bk_sb, in_=bkd.rearrange("(m p) -> p m", p=128))

            # ---- persistent activations ----
            qT = [persist.tile([128, N], F16, tag=f"qT{p}", name=f"qT{p}") for p in range(2)]
            kT = [persist.tile([128, N], F16, tag=f"kT{p}", name=f"kT{p}") for p in range(2)]
            ctxT = [persist.tile([128, N], F16, tag=f"ctxT{p}", name=f"ctxT{p}") for p in range(2)]
            # V for all 4 local heads: [nk-part, t, head*65 + (0:64 | ones)]
            v_all = persist.tile([128, NT, 4 * (HD + 1)], F16, tag="v")
            for h in range(4):
                nc.vector.memset(v_all[:, :, h * 65 + 64], 1.0)

            add, mult = mybir.AluOpType.add, mybir.AluOpType.mult

            # ---- projection building blocks ----
            # Each returns/consumes one PSUM accumulation group, small enough
            # to slot between attention groups without starving ACT.
            _xts = {}

            def load_x_chunk(pn):
                p, n = pn
                xts = []
                for k in range(KC):
                    xt = xp.tile([128, NQ], F16, tag="xt", name="xt")
                    nc.sync.dma_start(
                        out=xt,
                        in_=xT[k * 128:(k + 1) * 128, n * NQ:(n + 1) * NQ])
                    xts.append(xt)
                _xts[pn] = xts
                return xts

            def kq_group(p, n, wsb, bsb, dst):
                xts = _xts[(p, n)]
                ps = ppool.tile([128, NQ], F32, tag="pp", name="kqp")
                for k in range(KC):
                    nc.tensor.matmul(
                        ps, wsb[:, k, p * 128:(p + 1) * 128],
                        xts[k], start=(k == 0), stop=(k == KC - 1))
                nc.vector.tensor_tensor(
                    out=dst[p][:, n * NQ:(n + 1) * NQ], in0=ps,
                    in1=bsb[:, p:p + 1].broadcast_to([128, NQ]), op=add)

            def v_group(p, n, tl):
                xts = _xts[(p, n)]
                t = n * 4 + tl
                ps = ppool.tile([128, NQ], F32, tag="pp", name="vp")
                for k in range(KC):
                    nc.tensor.matmul(
                        ps[:, 0:128], xts[k][:, tl * 128:(tl + 1) * 128],
                        wv_sb[:, k, p * 128:(p + 1) * 128],
                        start=(k == 0), stop=(k == KC - 1))
                vv = v_all[:, t, :].rearrange("p (h c) -> p h c", c=65)
                nc.vector.tensor_copy(
                    out=vv[:, 2 * p:2 * p + 2, 0:64],
                    in_=ps[:, 0:128].rearrange("p (h c) -> p h c", c=64))

            def proj_fillers(p):
                # per n-chunk: K + 4 V + Q as 6 filler parcels; the x-chunk
                # DMAs are issued one n-chunk ahead so PE never head-of-line
                # blocks on a fresh load
                out = [lambda p=p: load_x_chunk((p, 0)),
                       lambda p=p: load_x_chunk((p, 1))]
                for n in range(NJ):
                    out.append(lambda p=p, n=n: kq_group(p, n, wk_sb, bk_sb, kT))
                    for tl in range(4):
                        out.append(lambda p=p, n=n, tl=tl: v_group(p, n, tl))
                    out.append(lambda p=p, n=n: (
                        kq_group(p, n, wq_sb, bq_sb, qT),
                        _xts.pop((p, n))))
                    if n + 2 < NJ:
                        out.insert(-4, lambda p=p, n=n: load_x_chunk((p, n + 2)))
                return out

            # pair-0 projections run up front (serial ACT-idle prefix
            # ~20us; attention needs all of kT0/v before it can start)
            for f in proj_fillers(0):
                f()

            # ---- attention (per head-pair p, nq chunk j of 256) ----
            # Software-pipelined: ctx matmuls for group g are emitted after
            # the S/exp of group g+1, so PE always has ready work while ACT
            # streams wide exps; heads alternate as the natural PSUM
            # ping-pong for the S tiles. The ctx PSUM bank is released by one
            # quick copy to SBUF; the reciprocal-normalize then runs fully
            # off the critical path on DVE/GpSimd.
            o_tiles = {}

            def out_proj_parcel(kp, m, eo, first):
                # one (m, eo) parcel of output-projection pass kp, all on the
                # double-buffered pp tag. Pass 0 copies, pass 1 adds + stores.
                if first and eo == 0:
                    o_tiles[m] = opool.tile([128, EMB], F32, tag="o", name="o")
                o = o_tiles[m]
                po = ppool.tile([128, NQ], F32, tag="pp", name="po")
                nc.tensor.matmul(
                    po, ctxT[kp][:, m * 128:(m + 1) * 128],
                    wo_sb[:, kp, eo * NQ:(eo + 1) * NQ],
                    start=True, stop=True)
                if first:
                    nc.vector.tensor_copy(o[:, eo * NQ:(eo + 1) * NQ], po)
                else:
                    nc.vector.tensor_tensor(
                        out=o[:, eo * NQ:(eo + 1) * NQ],
                        in0=o[:, eo * NQ:(eo + 1) * NQ], in1=po, op=add)
                if not first and eo == 1:
                    nc.sync.dma_start(
                        out=out_part[m * 128:(m + 1) * 128, :], in_=o)

            from collections import deque
            fillers = deque()

            for p in range(2):
                if p == 0:
                    # deferred wo load + pair-1 projections trickle through
                    # pair-0's attention window
                    fillers.append(lambda: nc.sync.dma_start(
                        out=wo_sb,
                        in_=woT.rearrange("(k p) e -> p k e", p=128)))
                    fillers.extend(proj_fillers(1))
                else:
                    # out-proj pass 0 (ctxT0 ready); pass-1 parcels are
                    # appended per-j as ctxT1 column windows complete
                    fillers.extend(
                        lambda m=m, eo=eo: out_proj_parcel(0, m, eo, True)
                        for m in range(NT) for eo in range(2))
                # pace: spread this window's fillers over its 8 j-iterations,
                # popping evenly between attention work items (8 per j)
                for j in range(NJA):
                    cps = [cpool.tile([HD + 1, NQA], F32, tag=f"c{h}",
                                      name=f"c{h}") for h in range(2)]

                    def s_mms(g, h):
                        lo = 64 * h
                        sp = spool.tile([128, len(g), NQA], F32,
                                        tag=f"s{h}", name=f"s{h}")
                        for i, t in enumerate(g):
                            nc.tensor.matmul(
                                sp[:, i, :],
                                kT[p][lo:lo + 64, t * 128:(t + 1) * 128],
                                qT[p][lo:lo + 64, j * NQA:(j + 1) * NQA],
                                start=True, stop=True)
                        return sp

                    def exp_act(sp, g, h):
                        e = epool.tile([128, len(g), NQA], F16,
                                       tag=f"e{h}", name=f"e{h}")
                        nc.scalar.activation(e, sp, FP.Exp, scale=SCALE)
                        return e

                    def ctx_mms(e, g, h):
                        hloc = 2 * p + h
                        for i, t in enumerate(g):
                            nc.tensor.matmul(
                                cps[h],
                                v_all[:, t, hloc * 65:(hloc + 1) * 65],
                                e[:, i, :],
                                start=(t == 0), stop=(t == NT - 1))

                    # interleave the two heads' group streams; ctx trails by
                    # one work item so PE always has ready matmuls queued.
                    # Filler parcels (projections / out-proj) are popped
                    # between work items, paced to spread over remaining j's.
                    work = []
                    for gi in range(max(len(T_GROUPS_H[0]), len(T_GROUPS_H[1]))):
                        for h in range(2):
                            if gi < len(T_GROUPS_H[h]):
                                work.append((T_GROUPS_H[h][gi], h))
                    n_pop = -(-len(fillers) // (NJA - j))  # ceil
                    prev = None
                    for wi, (g, h) in enumerate(work):
                        sp = s_mms(g, h)
                        cur = (exp_act(sp, g, h), g, h)
                        if prev is not None:
                            ctx_mms(*prev)
                        prev = cur
                        if fillers and wi < n_pop:
                            fillers.popleft()()
                    ctx_mms(*prev)
                    for _ in range(8, n_pop):
                        if fillers:
                            fillers.popleft()()

                    # normalize: ctx^T[0:64] * (1 / rowsum); rowsum in row 64.
                    # First copy out of PSUM (frees the ctx bank), then the
                    # slow reciprocal chain runs out of SBUF asynchronously.
                    for h in range(2):
                        cs = npool.tile([HD + 1, NQA], F32, tag="cs", name="cs")
                        nc.vector.tensor_copy(cs, cps[h])
                        # partition_broadcast reads physical partition 0, so
                        # stage the rowsum row there first
                        rs = npool.tile([1, NQA], F32, tag="rs", name="rs")
                        nc.vector.tensor_copy(rs, cs[64:65, :])
                        rb = npool.tile([64, NQA], F32, tag="rb", name="rb")
                        nc.gpsimd.partition_broadcast(rb, rs)
                        rc = npool.tile([64, NQA], F32, tag="rc", name="rc")
                        nc.vector.reciprocal(rc, rb)
                        nc.vector.tensor_tensor(
                            out=ctxT[p][h * 64:(h + 1) * 64,
                                        j * NQA:(j + 1) * NQA],
                            in0=cs[0:64, :], in1=rc, op=mult)
                    if p == 1:
                        # ctxT1 columns for this j are final -> out-proj
                        # pass-1 parcels for the covered m-chunks can run
                        for m in (2 * j, 2 * j + 1):
                            for eo in range(2):
                                fillers.append(
                                    lambda m=m, eo=eo:
                                    out_proj_parcel(1, m, eo, False))
            while fillers:
                fillers.popleft()()

    nc.compile()
    return nc


_NC_CACHE = {}


def _get_program():
    if "nc" not in _NC_CACHE:
        _NC_CACHE["nc"] = build_program()
    return _NC_CACHE["nc"]


def make_in_maps(x, Wq, bq, Wk, bk, Wv, bv, Wo, bo):
    x = np.asarray(x)
    xTs = [np.ascontiguousarray(x[b].T.astype(np.float16)) for b in range(B)]
    in_maps = []
    for c in range(N_CORES):
        b, hg = divmod(c, TP)
        sl = slice(hg * DQ, (hg + 1) * DQ)
        in_maps.append({
            "xT": xTs[b],
            "wqT": np.ascontiguousarray(np.asarray(Wq, np.float16)[sl, :].T),
            "wkT": np.ascontiguousarray(np.asarray(Wk, np.float16)[sl, :].T),
            "wvT": np.ascontiguousarray(np.asarray(Wv, np.float16)[sl, :].T),
            "woT": np.ascontiguousarray(np.asarray(Wo, np.float16)[:, sl].T),
            "bq_s": np.ascontiguousarray(np.asarray(bq, np.float32)[sl]),
            "bk_s": np.ascontiguousarray(np.asarray(bk, np.float32)[sl]),
        })
    return in_maps


def assemble_output(results, Wv_bias_term):
    out = np.empty((B, N, EMB), np.float32)
    for b in range(B):
        acc = results[b * TP]["out_part"].astype(np.float32)
        for g in range(1, TP):
            acc = acc + results[b * TP + g]["out_part"]
        out[b] = acc + Wv_bias_term
    return out


def kernel(x, Wq, bq, Wk, bk, Wv, bv, Wo, bo):
    nc = _get_program()
    in_maps = make_in_maps(x, Wq, bq, Wk, bk, Wv, bv, Wo, bo)
    res = run_bass_kernel_spmd(nc, in_maps, list(range(N_CORES)))
    bias_term = (np.asarray(bv, np.float32) @ np.asarray(Wo, np.float32).T
                 + np.asarray(bo, np.float32))
    return assemble_output(res.results, bias_term)



# revision 11
# speedup vs baseline: 1.2022x; 1.2022x over previous
"""Multi-head self-attention (b=2, n=2048, emb=1024, heads=16) on 8 trn2 cores.

Sharding: core c = (b, hg) with b = c // 4, hg = c % 4. Data parallel over
batch, tensor parallel over head-groups (4 heads / 256 emb-cols per core).
Each core computes Q/K/V projections for its heads, full attention for its
heads, and a partial output projection ctx_hg @ Wo[:, hg_slice].T of shape
[2048, 1024] (fp16). The host sums the 4 partials per batch and adds the
rank-1 bias term bv @ Wo.T + bo.

v2 redesign (ACT-bound target ~150-170us):
- nq attention chunk = 512 (4 j-chunks). S^T per (t, head-pair) computed as
  TWO row-tiled 64-contraction matmuls at tile_position (0,0)/(64,0) that
  run CONCURRENTLY on the PE (heads of a pair live in partitions 0-63 /
  64-127 of qT/kT), each writing one full PSUM bank [128, 512] f32.
- exp runs 1024-wide over the 2-bank S tile (both heads at once); the exp
  stream is the critical resource (128 x ~1.15us = 147us) so everything
  else is paced to hide under it.
- ctx^T accumulates [65, 512] per head over 16 nk-chunks; ones-column is
  FIRST (row 0 = softmax denominator, lands on physical partition 0 so
  gpsimd.partition_broadcast needs no staging copy). Normalize =
  copy (frees bank) + reciprocal_approx_fast on [1,512] + broadcast +
  one scalar_tensor_tensor multiply into fp16 ctxT.
- x is loaded ONCE (32KB/partition resident) and shared by both pairs'
  projections; V projection computed once for all 4 heads with a 256-wide
  moving operand.
- All projection/out-projection work is split into <=600ns parcels and
  deadline-scheduled into the per-t ACT slack so the PE never blocks the
  exp stream; j=0 of pair 0 interleaves K/V/Q chunk-wise with the S/exp
  stream as x chunks arrive from HBM.
- output partials stored/DMA'd as fp16.
"""

import os
import sys

for _p in ("/opt/trn_rl_repo", "/root/.axon_site/_ro/trn_rl_repo"):
    if os.path.isdir(_p) and _p not in sys.path:
        sys.path.append(_p)

import numpy as np

import concourse.bass as bass  # noqa: F401
import concourse.mybir as mybir
import concourse.tile as tile
from concourse import bacc
from concourse.bass_utils import run_bass_kernel_spmd

B, N, EMB, HEADS, HD = 2, 2048, 1024, 16, 64
N_CORES = 8
TP = 4                      # head-group shards per batch
DQ = EMB // TP              # 256 emb-cols (4 heads) per core
SCALE = HD ** -0.5          # 0.125

F32 = mybir.dt.float32
F16 = mybir.dt.float16
FP = mybir.ActivationFunctionType

NQ = 512                    # token chunk for projections and attention
NJ = N // NQ                # 4 chunks
NKC = 128                   # nk chunk (ctx contraction)
NT = N // NKC               # 16 nk chunks
KC = EMB // 128             # 8 contraction chunks for projections


def build_program():
    nc = bacc.Bacc("TRN2", target_bir_lowering=False, debug=False,
                   num_devices=N_CORES)

    xT = nc.dram_tensor("xT", [EMB, N], F16, kind="ExternalInput").ap()
    wqT = nc.dram_tensor("wqT", [EMB, DQ], F16, kind="ExternalInput").ap()
    wkT = nc.dram_tensor("wkT", [EMB, DQ], F16, kind="ExternalInput").ap()
    wvT = nc.dram_tensor("wvT", [EMB, DQ], F16, kind="ExternalInput").ap()
    woT = nc.dram_tensor("woT", [DQ, EMB], F16, kind="ExternalInput").ap()
    bqd = nc.dram_tensor("bq_s", [DQ], F32, kind="ExternalInput").ap()
    bkd = nc.dram_tensor("bk_s", [DQ], F32, kind="ExternalInput").ap()
    out_part = nc.dram_tensor("out_part", [N, EMB], F16,
                              kind="ExternalOutput").ap()

    add, mult = mybir.AluOpType.add, mybir.AluOpType.mult
    bypass = mybir.AluOpType.bypass

    with tile.TileContext(nc) as tc:
        with (
            tc.tile_pool(name="const", bufs=1) as const,
            tc.tile_pool(name="persist", bufs=1) as persist,
            tc.tile_pool(name="epool", bufs=4) as epool,
            tc.tile_pool(name="npool", bufs=2) as npool,
            tc.tile_pool(name="opool", bufs=NT) as opool,
            tc.tile_pool(name="o16pool", bufs=4) as o16pool,
            # PSUM budget (8 banks): s 2x2 + c 1+1 + pp 1x2 = 8
            tc.tile_pool(name="spool", bufs=2, space="PSUM") as spool,
            tc.tile_pool(name="cpool", bufs=1, space="PSUM") as cpool,
            tc.tile_pool(name="ppool", bufs=2, space="PSUM") as ppool,
        ):
            # ---- early exp table load (~2.7us) under the DMA prefix ----
            zt = const.tile([1, 1], F32, tag="zt", name="zt")
            nc.vector.memset(zt, 0.0)
            zo = const.tile([1, 1], F32, tag="zo", name="zo")
            nc.scalar.activation(zo, zt, FP.Exp)

            # ---- weights ----
            # DMA order matters at startup: the first K-projection group
            # needs wk[k] + x(0)[k] only, so those two streams interleave
            # per-k; wq/x(1), then wv/biases follow.
            wq_sb = const.tile([128, KC, DQ], F16, tag="wq", name="wq")
            wk_sb = const.tile([128, KC, DQ], F16, tag="wk", name="wk")
            wv_sb = const.tile([128, KC, DQ], F16, tag="wv", name="wv")
            wo_sb = const.tile([128, 2, EMB], F16, tag="wo", name="wo")  # deferred DMA
            bq_sb = const.tile([128, 2], F32, tag="bq", name="bq")
            bk_sb = const.tile([128, 2], F32, tag="bk", name="bk")

            # ---- persistent activations ----
            x_all = persist.tile([128, KC, NJ, NQ], F16, tag="x", name="x")
            qT = [persist.tile([128, N], F16, tag=f"qT{p}", name=f"qT{p}") for p in range(2)]
            kT = [persist.tile([128, N], F16, tag=f"kT{p}", name=f"kT{p}") for p in range(2)]
            ctxT = [persist.tile([128, N], F16, tag=f"ctxT{p}", name=f"ctxT{p}") for p in range(2)]
            # V for 4 local heads: [nk-part, t, head*65 + (0:64 | ones)]
            v_all = persist.tile([128, NT, 4 * (HD + 1)], F16, tag="v", name="v")
            for h in range(4):
                nc.vector.memset(v_all[:, :, h * 65 + 64], 1.0)

            def x_dma(n):
                for k in range(KC):
                    nc.sync.dma_start(
                        out=x_all[:, k, n, :],
                        in_=xT[k * 128:(k + 1) * 128, n * NQ:(n + 1) * NQ])

            # ---- projection parcels ----
            # kq_group(p, n): 8 accumulating MMs + bias-add, emitted as
            # 4x(2 MMs) + 1 DVE parcel so each slots into per-t ACT slack.
            _pp = {}

            def kq_mms(p, n, wsb, key, ks):
                if ks == 0:
                    _pp[key] = ppool.tile([128, NQ], F32, tag="pp", name="pp")
                ps = _pp[key]
                for k in (ks, ks + 1):
                    nc.tensor.matmul(
                        ps, wsb[:, k, p * 128:(p + 1) * 128],
                        x_all[:, k, n, :], start=(k == 0), stop=(k == KC - 1))

            def kq_fin(p, n, bsb, dst, key):
                ps = _pp.pop(key)
                nc.vector.tensor_tensor(
                    out=dst[p][:, n * NQ:(n + 1) * NQ], in0=ps,
                    in1=bsb[:, p:p + 1].broadcast_to([128, NQ]), op=add)

            def kq_parcels(p, n, wsb, bsb, dst, key):
                out = [lambda ks=ks: kq_mms(p, n, wsb, key, ks)
                       for ks in (0, 2, 4, 6)]
                out.append(lambda: kq_fin(p, n, bsb, dst, key))
                return out

            def v_mms(n, tl, key, ks):
                if ks == 0:
                    _pp[key] = ppool.tile([128, NQ], F32, tag="pp", name="pp")
                ps = _pp[key]
                for k in (ks, ks + 1):
                    nc.tensor.matmul(
                        ps[:, 0:256], x_all[:, k, n, tl * 128:(tl + 1) * 128],
                        wv_sb[:, k, :], start=(k == 0), stop=(k == KC - 1))

            def v_fin(n, tl, key):
                ps = _pp.pop(key)
                t = n * 4 + tl
                vv = v_all[:, t, :].rearrange("p (h c) -> p h c", c=65)
                nc.vector.tensor_copy(
                    out=vv[:, :, 0:64],
                    in_=ps[:, 0:256].rearrange("p (h c) -> p h c", c=64))

            def v_parcels(n, tl):
                key = ("v", n, tl)
                out = [lambda ks=ks: v_mms(n, tl, key, ks)
                       for ks in (0, 2, 4, 6)]
                out.append(lambda: v_fin(n, tl, key))
                return out

            # ---- out-projection parcels ----
            o_tiles = {}

            def out_proj_parcel(kp, m, eo):
                if kp == 0 and eo == 0:
                    o_tiles[m] = opool.tile([128, EMB], F32, tag="o", name="o")
                o = o_tiles[m]
                po = ppool.tile([128, NQ], F32, tag="pp", name="pp")
                nc.tensor.matmul(
                    po, ctxT[kp][:, m * 128:(m + 1) * 128],
                    wo_sb[:, kp, eo * NQ:(eo + 1) * NQ],
                    start=True, stop=True)
                if kp == 0:
                    nc.vector.tensor_copy(o[:, eo * NQ:(eo + 1) * NQ], po)
                else:
                    o16 = o_tiles[("f", m)]
                    nc.vector.tensor_tensor(
                        out=o16[:, eo * NQ:(eo + 1) * NQ],
                        in0=o[:, eo * NQ:(eo + 1) * NQ], in1=po, op=add)

            def out_proj_p1(m, eo):
                if eo == 0:
                    o_tiles[("f", m)] = o16pool.tile([128, EMB], F16, tag="o16", name="o16")
                out_proj_parcel(1, m, eo)
                if eo == 1:
                    o16 = o_tiles.pop(("f", m))
                    o_tiles.pop(m)
                    nc.sync.dma_start(
                        out=out_part[m * 128:(m + 1) * 128, :], in_=o16)

            # ---- attention machinery ----
            def s_pair(p, j, t):
                sg = spool.tile([128, 2, NQ], F32, tag="s", name="s")
                for h in range(2):
                    nc.tensor.matmul(
                        sg[:, h, :],
                        kT[p][64 * h:64 * h + 64, t * 128:(t + 1) * 128],
                        qT[p][64 * h:64 * h + 64, j * NQ:(j + 1) * NQ],
                        start=True, stop=True)
                return sg

            def exp_act(sg):
                e = epool.tile([128, 2, NQ], F16, tag="e", name="e")
                nc.scalar.activation(e, sg, FP.Exp, scale=SCALE)
                return e

            def ctx_pair(p, cps, e, t):
                for h in range(2):
                    hl = 2 * p + h
                    nc.tensor.matmul(
                        cps[h], v_all[:, t, hl * 65:(hl + 1) * 65],
                        e[:, h, :], start=(t == 0), stop=(t == NT - 1))

            def normalize(p, j, h, cps):
                # copy out of PSUM first (frees the ctx bank for next j);
                # rowsum sits in row 64 -> stage to partition 0 for the
                # gpsimd broadcast, reciprocal on the [1, 512] staged row.
                cs = npool.tile([65, NQ], F32, tag="cs", name="cs")
                nc.vector.tensor_copy(cs, cps[h])
                rs = npool.tile([1, NQ], F32, tag="rs", name="rs")
                nc.vector.tensor_copy(rs, cs[64:65, :])
                rc = npool.tile([1, NQ], F32, tag="rc", name="rc")
                nc.vector.reciprocal_approx_fast(rc, rs)
                rb = npool.tile([64, NQ], F32, tag="rb", name="rb")
                nc.gpsimd.partition_broadcast(rb, rc)
                nc.vector.scalar_tensor_tensor(
                    out=ctxT[p][h * 64:(h + 1) * 64, j * NQ:(j + 1) * NQ],
                    in0=cs[0:64, :], scalar=1.0, in1=rb,
                    op0=mult, op1=mult)

            # ---- schedule ----
            # Per (p, j): 16 t-iterations. Each iteration emits the S pair,
            # the 1024-wide exp, the previous t's ctx pair, then pops filler
            # parcels from `sched[iter]` (deadline-ordered small parcels).
            def attention_pair(p, sched, tail_sched=None):
                for j in range(NJ):
                    cps = [cpool.tile([65, NQ], F32, tag=f"c{h}", name=f"c{h}")
                           for h in range(2)]
                    prev = None
                    for t in range(NT):
                        it = j * NT + t
                        for f in sched.get(("pre", it), ()):
                            f()
                        sg = s_pair(p, j, t)
                        e = exp_act(sg)
                        if prev is not None:
                            ctx_pair(p, cps, *prev)
                        prev = (e, t)
                        for f in sched.get(it, ()):
                            f()
                    ctx_pair(p, cps, *prev)
                    for h in range(2):
                        normalize(p, j, h, cps)
                    if tail_sched is not None:
                        for f in tail_sched.get(j, ()):
                            f()

            # ---------- pair-0 window ----------
            # prefix: x chunk 0/1 in flight; K0(0), Q0(0), V(0, tl=0) emitted
            # before the t-loop (S(0..3) needs K0(0)+Q0(0); ctx(0) needs
            # V(0,0)).
            for k in range(KC):
                nc.sync.dma_start(out=wk_sb[:, k, :], in_=wkT.rearrange(
                    "(k p) d -> k p d", p=128)[k])
                nc.sync.dma_start(
                    out=x_all[:, k, 0, :],
                    in_=xT[k * 128:(k + 1) * 128, 0:NQ])
            nc.sync.dma_start(out=bk_sb, in_=bkd.rearrange("(m p) -> p m", p=128))
            nc.sync.dma_start(out=bq_sb, in_=bqd.rearrange("(m p) -> p m", p=128))
            for k in range(KC):
                nc.sync.dma_start(out=wq_sb[:, k, :], in_=wqT.rearrange(
                    "(k p) d -> k p d", p=128)[k])
                nc.sync.dma_start(
                    out=x_all[:, k, 1, :],
                    in_=xT[k * 128:(k + 1) * 128, NQ:2 * NQ])
            for k in range(KC):
                nc.sync.dma_start(out=wv_sb[:, k, :], in_=wvT.rearrange(
                    "(k p) d -> k p d", p=128)[k])
            for f in kq_parcels(0, 0, wk_sb, bk_sb, kT, ("k0", 0)):
                f()
            for f in kq_parcels(0, 0, wq_sb, bq_sb, qT, ("q0", 0)):
                f()
            for f in v_parcels(0, 0):
                f()

            # j=0 inline schedule. Whole groups per slot (keeps <=2 pp
            # accumulation groups open at any time):
            #   K0(n) at slot 4n-3 (S(4n) reads kT at iter 4n)
            #   V(n,tl) at slot 4n+tl-1 (ctx(t) runs at iter t+1)
            #   Q0(n) at slot 16n-4 (j=n's S reads qT window n)
            # j=0 is PE-bound (projections + x DMA); j=1..3 have ~500ns/iter
            # of PE slack under the exp stream, where the pair-1 K/Q
            # projections trickle through as 2-MM parcels.
            sched0 = {}

            def put(it, f):
                sched0.setdefault(it, []).append(f)

            for tl in (1, 2, 3):            # V(0) tl=1..3 -> slots 0,1,2
                for f in v_parcels(0, tl):
                    put(tl - 1, f)
            for n in (1, 2, 3):
                if n + 1 < NJ:
                    put(4 * n - 4, lambda n=n: x_dma(n + 1))
                for f in kq_parcels(0, n, wk_sb, bk_sb, kT, ("k0", n)):
                    put(4 * n - 3, f)
                for tl in range(4):
                    for f in v_parcels(n, tl):
                        put(4 * n + tl - 1, f)
                for f in kq_parcels(0, n, wq_sb, bq_sb, qT, ("q0", n)):
                    put(16 * n - 4, f)
            put(13, lambda: nc.sync.dma_start(
                out=wo_sb, in_=woT.rearrange("(k p) e -> p k e", p=128)))
            # pair-1 projections spread over j=1..3 free iters
            free_iters = [it for it in range(16, 64)
                          if len(sched0.get(it, [])) == 0]
            fill = []
            for n in range(NJ):
                fill += kq_parcels(1, n, wk_sb, bk_sb, kT, ("k1", n))
                fill += kq_parcels(1, n, wq_sb, bq_sb, qT, ("q1", n))
            for i, f in enumerate(fill):
                put(free_iters[i * len(free_iters) // len(fill)], f)

            attention_pair(0, sched0)

            # ---------- pair-1 window ----------
            # out-proj pass 0 (32 parcels, iters 0..31); pass 1 for j at
            # 8 iters inside window j+1; j=3's at the tail.
            sched1 = {}
            tail1 = {}
            for m in range(NT):
                for eo in range(2):
                    sched1.setdefault(2 * m + eo, []).append(
                        lambda m=m, eo=eo: out_proj_parcel(0, m, eo))
            for j in range(NJ):
                fs = []
                for m in range(4 * j, 4 * j + 4):
                    for eo in range(2):
                        fs.append(lambda m=m, eo=eo: out_proj_p1(m, eo))
                if j < NJ - 1:
                    base = 33 + 8 * j
                    for i, f in enumerate(fs):
                        sched1.setdefault(base + i, []).append(f)
                else:
                    tail1[j] = fs
            attention_pair(1, sched1, tail1)

    nc.compile()
    return nc


_NC_CACHE = {}


def _get_program():
    if "nc" not in _NC_CACHE:
        _NC_CACHE["nc"] = build_program()
    return _NC_CACHE["nc"]


def make_in_maps(x, Wq, bq, Wk, bk, Wv, bv, Wo, bo):
    x = np.asarray(x)
    xTs = [np.ascontiguousarray(x[b].T.astype(np.float16)) for b in range(B)]
    in_maps = []
    for c in range(N_CORES):
        b, hg = divmod(c, TP)
        sl = slice(hg * DQ, (hg + 1) * DQ)
        in_maps.append({
            "xT": xTs[b],
            "wqT": np.ascontiguousarray(np.asarray(Wq, np.float16)[sl, :].T),
            "wkT": np.ascontiguousarray(np.asarray(Wk, np.float16)[sl, :].T),
            "wvT": np.ascontiguousarray(np.asarray(Wv, np.float16)[sl, :].T),
            "woT": np.ascontiguousarray(np.asarray(Wo, np.float16)[:, sl].T),
            "bq_s": np.ascontiguousarray(np.asarray(bq, np.float32)[sl]),
            "bk_s": np.ascontiguousarray(np.asarray(bk, np.float32)[sl]),
        })
    return in_maps


def assemble_output(results, Wv_bias_term):
    out = np.empty((B, N, EMB), np.float32)
    for b in range(B):
        acc = results[b * TP]["out_part"].astype(np.float32)
        for g in range(1, TP):
            acc = acc + results[b * TP + g]["out_part"].astype(np.float32)
        out[b] = acc + Wv_bias_term
    return out


def kernel(x, Wq, bq, Wk, bk, Wv, bv, Wo, bo):
    nc = _get_program()
    in_maps = make_in_maps(x, Wq, bq, Wk, bk, Wv, bv, Wo, bo)
    res = run_bass_kernel_spmd(nc, in_maps, list(range(N_CORES)))
    bias_term = (np.asarray(bv, np.float32) @ np.asarray(Wo, np.float32).T
                 + np.asarray(bo, np.float32))
    return assemble_output(res.results, bias_term)


# revision 17
# speedup vs baseline: 1.2457x; 1.0362x over previous
"""Multi-head self-attention (b=2, n=2048, emb=1024, heads=16) on 8 trn2 cores.

Sharding: core c = (b, hg) with b = c // 4, hg = c % 4. Data parallel over
batch, tensor parallel over head-groups (4 heads / 256 emb-cols per core).
Each core computes Q/K/V projections for its heads, full attention for its
heads, and a partial output projection ctx_hg @ Wo[:, hg_slice].T of shape
[2048, 1024] (fp16). The host sums the 4 partials per batch and adds the
rank-1 bias term bv @ Wo.T + bo.

v2 redesign (ACT-bound target ~150-170us):
- nq attention chunk = 512 (4 j-chunks). S^T per (t, head-pair) computed as
  TWO row-tiled 64-contraction matmuls at tile_position (0,0)/(64,0) that
  run CONCURRENTLY on the PE (heads of a pair live in partitions 0-63 /
  64-127 of qT/kT), each writing one full PSUM bank [128, 512] f32.
- exp runs 1024-wide over the 2-bank S tile (both heads at once); the exp
  stream is the critical resource (128 x ~1.15us = 147us) so everything
  else is paced to hide under it.
- ctx^T accumulates [65, 512] per head over 16 nk-chunks; ones-column is
  FIRST (row 0 = softmax denominator, lands on physical partition 0 so
  gpsimd.partition_broadcast needs no staging copy). Normalize =
  copy (frees bank) + reciprocal_approx_fast on [1,512] + broadcast +
  one scalar_tensor_tensor multiply into fp16 ctxT.
- x is loaded ONCE (32KB/partition resident) and shared by both pairs'
  projections; V projection computed once for all 4 heads with a 256-wide
  moving operand.
- All projection/out-projection work is split into <=600ns parcels and
  deadline-scheduled into the per-t ACT slack so the PE never blocks the
  exp stream; j=0 of pair 0 interleaves K/V/Q chunk-wise with the S/exp
  stream as x chunks arrive from HBM.
- output partials stored/DMA'd as fp16.
"""

import os
import sys

for _p in ("/opt/trn_rl_repo", "/root/.axon_site/_ro/trn_rl_repo"):
    if os.path.isdir(_p) and _p not in sys.path:
        sys.path.append(_p)

import numpy as np

import concourse.bass as bass  # noqa: F401
import concourse.mybir as mybir
import concourse.tile as tile
from concourse import bacc
from concourse.bass_utils import run_bass_kernel_spmd

B, N, EMB, HEADS, HD = 2, 2048, 1024, 16, 64
N_CORES = 8
TP = 4                      # head-group shards per batch
DQ = EMB // TP              # 256 emb-cols (4 heads) per core
SCALE = HD ** -0.5          # 0.125

F32 = mybir.dt.float32
F16 = mybir.dt.float16
FP = mybir.ActivationFunctionType

NQ = 512                    # token chunk for projections and attention
NJ = N // NQ                # 4 chunks
NKC = 128                   # nk chunk (ctx contraction)
NT = N // NKC               # 16 nk chunks
KC = EMB // 128             # 8 contraction chunks for projections


def build_program():
    nc = bacc.Bacc("TRN2", target_bir_lowering=False, debug=False,
                   num_devices=N_CORES)

    xT = nc.dram_tensor("xT", [EMB, N], F16, kind="ExternalInput").ap()
    wqT = nc.dram_tensor("wqT", [EMB, DQ], F16, kind="ExternalInput").ap()
    wkT = nc.dram_tensor("wkT", [EMB, DQ], F16, kind="ExternalInput").ap()
    wvT = nc.dram_tensor("wvT", [EMB, DQ], F16, kind="ExternalInput").ap()
    woT = nc.dram_tensor("woT", [DQ, EMB], F16, kind="ExternalInput").ap()
    bqd = nc.dram_tensor("bq_s", [DQ], F32, kind="ExternalInput").ap()
    bkd = nc.dram_tensor("bk_s", [DQ], F32, kind="ExternalInput").ap()
    out_part = nc.dram_tensor("out_part", [N, EMB], F16,
                              kind="ExternalOutput").ap()

    add, mult = mybir.AluOpType.add, mybir.AluOpType.mult
    bypass = mybir.AluOpType.bypass

    with tile.TileContext(nc) as tc:
        with (
            tc.tile_pool(name="const", bufs=1) as const,
            tc.tile_pool(name="persist", bufs=1) as persist,
            tc.tile_pool(name="epool", bufs=4) as epool,
            tc.tile_pool(name="npool", bufs=2) as npool,
            tc.tile_pool(name="opool", bufs=NT) as opool,
            tc.tile_pool(name="o16pool", bufs=4) as o16pool,
            # PSUM budget (8 banks): s 2x2 + c 1+1 + pp 1x2 = 8
            tc.tile_pool(name="spool", bufs=2, space="PSUM") as spool,
            tc.tile_pool(name="cpool", bufs=1, space="PSUM") as cpool,
            tc.tile_pool(name="ppool", bufs=2, space="PSUM") as ppool,
        ):
            # ---- early exp table load (~2.7us) under the DMA prefix ----
            zt = const.tile([1, 1], F32, tag="zt", name="zt")
            nc.vector.memset(zt, 0.0)
            zo = const.tile([1, 1], F32, tag="zo", name="zo")
            nc.scalar.activation(zo, zt, FP.Exp)

            # ---- weights ----
            # DMA order matters at startup: the first K-projection group
            # needs wk[k] + x(0)[k] only, so those two streams interleave
            # per-k; wq/x(1), then wv/biases follow.
            wq_sb = const.tile([128, KC, DQ], F16, tag="wq", name="wq")
            wk_sb = const.tile([128, KC, DQ], F16, tag="wk", name="wk")
            wv_sb = const.tile([128, KC, DQ], F16, tag="wv", name="wv")
            wo_sb = const.tile([128, 2, EMB], F16, tag="wo", name="wo")  # deferred DMA
            bq_sb = const.tile([128, 2], F32, tag="bq", name="bq")
            bk_sb = const.tile([128, 2], F32, tag="bk", name="bk")

            # ---- persistent activations ----
            x_all = persist.tile([128, KC, NJ, NQ], F16, tag="x", name="x")
            qT = [persist.tile([128, N], F16, tag=f"qT{p}", name=f"qT{p}") for p in range(2)]
            kT = [persist.tile([128, N], F16, tag=f"kT{p}", name=f"kT{p}") for p in range(2)]
            ctxT = [persist.tile([128, N], F16, tag=f"ctxT{p}", name=f"ctxT{p}") for p in range(2)]
            # V for 4 local heads: [nk-part, t, head*65 + (0:64 | ones)]
            v_all = persist.tile([128, NT, 4 * (HD + 1)], F16, tag="v", name="v")
            for h in range(4):
                nc.vector.memset(v_all[:, :, h * 65 + 64], 1.0)

            def x_dma(n):
                # x chunks ride the (startup-idle) GpSimd/Vector DMA queues,
                # k-granular so K/V/Q groups start on partial arrivals;
                # weights stream in parallel on the Sync queue.
                for k in range(KC):
                    eng = nc.gpsimd if k % 2 == 0 else nc.sync
                    eng.dma_start(
                        out=x_all[:, k, n, :],
                        in_=xT[k * 128:(k + 1) * 128, n * NQ:(n + 1) * NQ])

            # ---- projection parcels ----
            # kq_group(p, n): 8 accumulating MMs + bias-add, emitted as
            # 4x(2 MMs) + 1 DVE parcel so each slots into per-t ACT slack.
            _pp = {}

            def kq_mms(p, n, wsb, key, ks):
                if ks == 0:
                    _pp[key] = ppool.tile([128, NQ], F32, tag="pp", name="pp")
                ps = _pp[key]
                for k in (ks, ks + 1):
                    nc.tensor.matmul(
                        ps, wsb[:, k, p * 128:(p + 1) * 128],
                        x_all[:, k, n, :], start=(k == 0), stop=(k == KC - 1))

            def kq_fin(p, n, bsb, dst, key):
                ps = _pp.pop(key)
                nc.vector.tensor_tensor(
                    out=dst[p][:, n * NQ:(n + 1) * NQ], in0=ps,
                    in1=bsb[:, p:p + 1].broadcast_to([128, NQ]), op=add)

            def kq_parcels(p, n, wsb, bsb, dst, key):
                out = [lambda ks=ks: kq_mms(p, n, wsb, key, ks)
                       for ks in (0, 2, 4, 6)]
                out.append(lambda: kq_fin(p, n, bsb, dst, key))
                return out

            def v_mms(n, tl, key, ks):
                if ks == 0:
                    _pp[key] = ppool.tile([128, NQ], F32, tag="pp", name="pp")
                ps = _pp[key]
                for k in (ks, ks + 1):
                    nc.tensor.matmul(
                        ps[:, 0:256], x_all[:, k, n, tl * 128:(tl + 1) * 128],
                        wv_sb[:, k, :], start=(k == 0), stop=(k == KC - 1))

            def v_fin(n, tl, key):
                ps = _pp.pop(key)
                t = n * 4 + tl
                vv = v_all[:, t, :].rearrange("p (h c) -> p h c", c=65)
                nc.vector.tensor_copy(
                    out=vv[:, :, 0:64],
                    in_=ps[:, 0:256].rearrange("p (h c) -> p h c", c=64))

            def v_parcels(n, tl):
                key = ("v", n, tl)
                out = [lambda ks=ks: v_mms(n, tl, key, ks)
                       for ks in (0, 2, 4, 6)]
                out.append(lambda: v_fin(n, tl, key))
                return out

            # ---- out-projection parcels ----
            o_tiles = {}

            def out_proj_parcel(kp, m, eo):
                if kp == 0 and eo == 0:
                    o_tiles[m] = opool.tile([128, EMB], F32, tag="o", name="o")
                o = o_tiles[m]
                po = ppool.tile([128, NQ], F32, tag="pp", name="pp")
                nc.tensor.matmul(
                    po, ctxT[kp][:, m * 128:(m + 1) * 128],
                    wo_sb[:, kp, eo * NQ:(eo + 1) * NQ],
                    start=True, stop=True)
                if kp == 0:
                    nc.vector.tensor_copy(o[:, eo * NQ:(eo + 1) * NQ], po)
                else:
                    o16 = o_tiles[("f", m)]
                    nc.vector.tensor_tensor(
                        out=o16[:, eo * NQ:(eo + 1) * NQ],
                        in0=o[:, eo * NQ:(eo + 1) * NQ], in1=po, op=add)

            def out_proj_p1(m, eo):
                if eo == 0:
                    o_tiles[("f", m)] = o16pool.tile([128, EMB], F16, tag="o16", name="o16")
                out_proj_parcel(1, m, eo)
                if eo == 1:
                    o16 = o_tiles.pop(("f", m))
                    o_tiles.pop(m)
                    nc.sync.dma_start(
                        out=out_part[m * 128:(m + 1) * 128, :], in_=o16)

            # ---- attention machinery ----
            def s_pair(p, j, t):
                sg = spool.tile([128, 2, NQ], F32, tag="s", name="s")
                for h in range(2):
                    nc.tensor.matmul(
                        sg[:, h, :],
                        kT[p][64 * h:64 * h + 64, t * 128:(t + 1) * 128],
                        qT[p][64 * h:64 * h + 64, j * NQ:(j + 1) * NQ],
                        start=True, stop=True)
                return sg

            def exp_act(sg):
                e = epool.tile([128, 2, NQ], F16, tag="e", name="e")
                nc.scalar.activation(e, sg, FP.Exp, scale=SCALE)
                return e

            def ctx_pair(p, cps, e, t):
                for h in range(2):
                    hl = 2 * p + h
                    nc.tensor.matmul(
                        cps[h], v_all[:, t, hl * 65:(hl + 1) * 65],
                        e[:, h, :], start=(t == 0), stop=(t == NT - 1))

            def normalize(p, j, h, cps):
                # copy out of PSUM first (frees the ctx bank for next j);
                # rowsum sits in row 64 -> stage to partition 0 for the
                # gpsimd broadcast, reciprocal on the [1, 512] staged row.
                cs = npool.tile([65, NQ], F32, tag="cs", name="cs")
                nc.vector.tensor_copy(cs, cps[h])
                rs = npool.tile([1, NQ], F32, tag="rs", name="rs")
                nc.vector.tensor_copy(rs, cs[64:65, :])
                rc = npool.tile([1, NQ], F32, tag="rc", name="rc")
                nc.vector.reciprocal_approx_fast(rc, rs)
                rb = npool.tile([64, NQ], F32, tag="rb", name="rb")
                nc.gpsimd.partition_broadcast(rb, rc)
                nc.vector.scalar_tensor_tensor(
                    out=ctxT[p][h * 64:(h + 1) * 64, j * NQ:(j + 1) * NQ],
                    in0=cs[0:64, :], scalar=1.0, in1=rb,
                    op0=mult, op1=mult)

            # ---- schedule ----
            # One flat software pipeline over all 128 (p, j, t) iterations:
            # iteration g emits S-pair(g), exp(g), then ctx-pair(g-2) (lag 2
            # so ctx never waits on the exp semaphore), then filler parcels
            # from sched[g]. j/p boundaries are crossed seamlessly; the
            # normalize for j is emitted right after its last ctx pair.
            sched = {}

            def put(g, f):
                sched.setdefault(g, []).append(f)

            cps_by = {}
            pend = []

            def emit_ctx(g2):
                p2, r2 = divmod(g2, 64)
                j2, t2 = divmod(r2, 16)
                if t2 == 0:
                    cps_by[(p2, j2)] = [
                        cpool.tile([65, NQ], F32, tag=f"c{h}", name=f"c{h}")
                        for h in range(2)]
                e2 = pend.pop(0)
                ctx_pair(p2, cps_by[(p2, j2)], e2, t2)
                if t2 == NT - 1:
                    cps2 = cps_by.pop((p2, j2))
                    for h in range(2):
                        normalize(p2, j2, h, cps2)

            # ---------- prefix ----------
            # Sync queue: monolithic weight DMAs; GpSimd/Vector queues: x.
            nc.sync.dma_start(out=wk_sb, in_=wkT.rearrange(
                "(k p) d -> p k d", p=128))
            nc.sync.dma_start(out=wq_sb, in_=wqT.rearrange(
                "(k p) d -> p k d", p=128))
            nc.sync.dma_start(out=wv_sb, in_=wvT.rearrange(
                "(k p) d -> p k d", p=128))
            nc.sync.dma_start(out=bk_sb, in_=bkd.rearrange("(m p) -> p m", p=128))
            nc.sync.dma_start(out=bq_sb, in_=bqd.rearrange("(m p) -> p m", p=128))
            for n in range(NJ):
                x_dma(n)
            for f in kq_parcels(0, 0, wk_sb, bk_sb, kT, ("k0", 0)):
                f()
            for f in kq_parcels(0, 0, wq_sb, bq_sb, qT, ("q0", 0)):
                f()

            # ---------- filler schedule ----------
            # pair-0 window (g 0..63):
            #   V(n, tl) whole-group at g=4n+tl+1 (ctx(t) runs at g=t+2)
            #   K0(n) parcels at g=4n-4..4n-1; Q0(n) parcels at g=16n-5..
            for n in range(NJ):
                for tl in range(4):
                    for f in v_parcels(n, tl):
                        put(4 * n + tl + 1, f)
            for n in (1, 2, 3):
                for i, f in enumerate(kq_parcels(0, n, wk_sb, bk_sb, kT,
                                                 ("k0", n))):
                    put(4 * n - 4 + min(i, 3), f)
                for i, f in enumerate(kq_parcels(0, n, wq_sb, bq_sb, qT,
                                                 ("q0", n))):
                    put(16 * n - 5 + i, f)
            put(13, lambda: nc.sync.dma_start(
                out=wo_sb, in_=woT.rearrange("(k p) e -> p k e", p=128)))
            free_iters = [g for g in range(17, 64)
                          if len(sched.get(g, [])) == 0]
            fill = []
            for n in range(NJ):
                fill += kq_parcels(1, n, wk_sb, bk_sb, kT, ("k1", n))
                fill += kq_parcels(1, n, wq_sb, bq_sb, qT, ("q1", n))
            for i, f in enumerate(fill):
                put(free_iters[i * len(free_iters) // len(fill)], f)
            # pair-1 window (g 64..127): out-proj pass 0 at g=64..95;
            # pass 1 for j at g=64+16(j+1)+2.. (8 parcels); j=3 in the tail.
            for m in range(NT):
                for eo in range(2):
                    put(64 + 2 * m + eo,
                        lambda m=m, eo=eo: out_proj_parcel(0, m, eo))
            for j in range(NJ - 1):
                fs = []
                for m in range(4 * j, 4 * j + 4):
                    for eo in range(2):
                        fs.append(lambda m=m, eo=eo: out_proj_p1(m, eo))
                for i, f in enumerate(fs):
                    put(64 + 16 * (j + 1) + 2 + i, f)

            # ---------- main pipeline ----------
            for g in range(128):
                p, r = divmod(g, 64)
                j, t = divmod(r, 16)
                sg = s_pair(p, j, t)
                pend.append(exp_act(sg))
                if g >= 2:
                    emit_ctx(g - 2)
                for f in sched.get(g, ()):
                    f()

            # ---------- tail: ctx/normalize for (p1, j3, t14..15) ----------
            # then pipelined per-m normalize-chunk + out-proj + DMA.
            p3, j3 = 1, NJ - 1
            cps3 = None
            for g2 in (126, 127):
                t2 = g2 % 16
                e2 = pend.pop(0)
                ctx_pair(p3, cps_by[(p3, j3)], e2, t2)
            cps3 = cps_by.pop((p3, j3))
            css, rbs = [], []
            for h in range(2):
                cs = npool.tile([65, NQ], F32, tag="cs", name="cs")
                nc.vector.tensor_copy(cs, cps3[h])
                rs = npool.tile([1, NQ], F32, tag="rs", name="rs")
                nc.vector.tensor_copy(rs, cs[64:65, :])
                rc = npool.tile([1, NQ], F32, tag="rc", name="rc")
                nc.vector.reciprocal_approx_fast(rc, rs)
                rb = npool.tile([64, NQ], F32, tag="rb", name="rb")
                nc.gpsimd.partition_broadcast(rb, rc)
                css.append(cs)
                rbs.append(rb)
            for m in range(4 * j3, 4 * j3 + 4):
                mo = (m - 4 * j3) * 128
                for h in range(2):
                    nc.vector.scalar_tensor_tensor(
                        out=ctxT[p3][h * 64:(h + 1) * 64,
                                     j3 * NQ + mo:j3 * NQ + mo + 128],
                        in0=css[h][0:64, mo:mo + 128], scalar=1.0,
                        in1=rbs[h][:, mo:mo + 128], op0=mult, op1=mult)
                out_proj_p1(m, 0)
                out_proj_p1(m, 1)

    nc.compile()
    return nc


_NC_CACHE = {}


def _get_program():
    if "nc" not in _NC_CACHE:
        _NC_CACHE["nc"] = build_program()
    return _NC_CACHE["nc"]


def make_in_maps(x, Wq, bq, Wk, bk, Wv, bv, Wo, bo):
    x = np.asarray(x)
    xTs = [np.ascontiguousarray(x[b].T.astype(np.float16)) for b in range(B)]
    in_maps = []
    for c in range(N_CORES):
        b, hg = divmod(c, TP)
        sl = slice(hg * DQ, (hg + 1) * DQ)
        in_maps.append({
            "xT": xTs[b],
            "wqT": np.ascontiguousarray(np.asarray(Wq, np.float16)[sl, :].T),
            "wkT": np.ascontiguousarray(np.asarray(Wk, np.float16)[sl, :].T),
            "wvT": np.ascontiguousarray(np.asarray(Wv, np.float16)[sl, :].T),
            "woT": np.ascontiguousarray(np.asarray(Wo, np.float16)[:, sl].T),
            "bq_s": np.ascontiguousarray(np.asarray(bq, np.float32)[sl]),
            "bk_s": np.ascontiguousarray(np.asarray(bk, np.float32)[sl]),
        })
    return in_maps


def assemble_output(results, Wv_bias_term):
    out = np.empty((B, N, EMB), np.float32)
    for b in range(B):
        acc = results[b * TP]["out_part"].astype(np.float32)
        for g in range(1, TP):
            acc = acc + results[b * TP + g]["out_part"].astype(np.float32)
        out[b] = acc + Wv_bias_term
    return out


def kernel(x, Wq, bq, Wk, bk, Wv, bv, Wo, bo):
    nc = _get_program()
    in_maps = make_in_maps(x, Wq, bq, Wk, bk, Wv, bv, Wo, bo)
    res = run_bass_kernel_spmd(nc, in_maps, list(range(N_CORES)))
    bias_term = (np.asarray(bv, np.float32) @ np.asarray(Wo, np.float32).T
                 + np.asarray(bo, np.float32))
    return assemble_output(res.results, bias_term)


# revision 20
# speedup vs baseline: 1.3192x; 1.0590x over previous
"""Multi-head self-attention (b=2, n=2048, emb=1024, heads=16) on 8 trn2 cores.

Sharding: core c = (b, hg) with b = c // 4, hg = c % 4. Data parallel over
batch, tensor parallel over head-groups (4 heads / 256 emb-cols per core).
Each core computes Q/K/V projections for its heads, full attention for its
heads, and a partial output projection ctx_hg @ Wo[:, hg_slice].T of shape
[2048, 1024] (fp16). The host sums the 4 partials per batch and adds the
rank-1 bias term bv @ Wo.T + bo.

v2 redesign (ACT-bound target ~150-170us):
- nq attention chunk = 512 (4 j-chunks). S^T per (t, head-pair) computed as
  TWO row-tiled 64-contraction matmuls at tile_position (0,0)/(64,0) that
  run CONCURRENTLY on the PE (heads of a pair live in partitions 0-63 /
  64-127 of qT/kT), each writing one full PSUM bank [128, 512] f32.
- exp runs 1024-wide over the 2-bank S tile (both heads at once); the exp
  stream is the critical resource (128 x ~1.15us = 147us) so everything
  else is paced to hide under it.
- ctx^T accumulates [65, 512] per head over 16 nk-chunks; ones-column is
  FIRST (row 0 = softmax denominator, lands on physical partition 0 so
  gpsimd.partition_broadcast needs no staging copy). Normalize =
  copy (frees bank) + reciprocal_approx_fast on [1,512] + broadcast +
  one scalar_tensor_tensor multiply into fp16 ctxT.
- x is loaded ONCE (32KB/partition resident) and shared by both pairs'
  projections; V projection computed once for all 4 heads with a 256-wide
  moving operand.
- All projection/out-projection work is split into <=600ns parcels and
  deadline-scheduled into the per-t ACT slack so the PE never blocks the
  exp stream; j=0 of pair 0 interleaves K/V/Q chunk-wise with the S/exp
  stream as x chunks arrive from HBM.
- output partials stored/DMA'd as fp16.
"""

import os
import sys

for _p in ("/opt/trn_rl_repo", "/root/.axon_site/_ro/trn_rl_repo"):
    if os.path.isdir(_p) and _p not in sys.path:
        sys.path.append(_p)

import numpy as np

import concourse.bass as bass  # noqa: F401
import concourse.mybir as mybir
import concourse.tile as tile
from concourse import bacc
from concourse.bass_utils import run_bass_kernel_spmd

B, N, EMB, HEADS, HD = 2, 2048, 1024, 16, 64
N_CORES = 8
TP = 4                      # head-group shards per batch
DQ = EMB // TP              # 256 emb-cols (4 heads) per core
SCALE = HD ** -0.5          # 0.125

F32 = mybir.dt.float32
F16 = mybir.dt.float16
FP = mybir.ActivationFunctionType

NQ = 512                    # token chunk for projections and attention
NJ = N // NQ                # 4 chunks
NKC = 128                   # nk chunk (ctx contraction)
NT = N // NKC               # 16 nk chunks
KC = EMB // 128             # 8 contraction chunks for projections


def build_program():
    nc = bacc.Bacc("TRN2", target_bir_lowering=False, debug=False,
                   num_devices=N_CORES)

    xT = nc.dram_tensor("xT", [EMB, N], F16, kind="ExternalInput").ap()
    wqT = nc.dram_tensor("wqT", [EMB, DQ], F16, kind="ExternalInput").ap()
    wkT = nc.dram_tensor("wkT", [EMB, DQ], F16, kind="ExternalInput").ap()
    wvT = nc.dram_tensor("wvT", [EMB, DQ], F16, kind="ExternalInput").ap()
    woT = nc.dram_tensor("woT", [DQ, EMB], F16, kind="ExternalInput").ap()
    bqd = nc.dram_tensor("bq_s", [DQ], F32, kind="ExternalInput").ap()
    bkd = nc.dram_tensor("bk_s", [DQ], F32, kind="ExternalInput").ap()
    out_part = nc.dram_tensor("out_part", [N, EMB], F16,
                              kind="ExternalOutput").ap()

    add, mult = mybir.AluOpType.add, mybir.AluOpType.mult
    bypass = mybir.AluOpType.bypass

    with tile.TileContext(nc) as tc:
        with (
            tc.tile_pool(name="const", bufs=1) as const,
            tc.tile_pool(name="persist", bufs=1) as persist,
            tc.tile_pool(name="epool", bufs=4) as epool,
            tc.tile_pool(name="npool", bufs=2) as npool,
            tc.tile_pool(name="opool", bufs=NT) as opool,
            tc.tile_pool(name="o16pool", bufs=4) as o16pool,
            # PSUM budget (8 banks): s 2x2 + c 1+1 + pp 1x2 = 8
            tc.tile_pool(name="spool", bufs=2, space="PSUM") as spool,
            tc.tile_pool(name="cpool", bufs=1, space="PSUM") as cpool,
            tc.tile_pool(name="ppool", bufs=2, space="PSUM") as ppool,
        ):
            # ---- early exp table load (~2.7us) under the DMA prefix ----
            zt = const.tile([1, 1], F32, tag="zt", name="zt")
            nc.vector.memset(zt, 0.0)
            zo = const.tile([1, 1], F32, tag="zo", name="zo")
            nc.scalar.activation(zo, zt, FP.Exp)

            # ---- weights ----
            # DMA order matters at startup: the first K-projection group
            # needs wk[k] + x(0)[k] only, so those two streams interleave
            # per-k; wq/x(1), then wv/biases follow.
            wq_sb = const.tile([128, KC, DQ], F16, tag="wq", name="wq")
            wk_sb = const.tile([128, KC, DQ], F16, tag="wk", name="wk")
            wv_sb = const.tile([128, KC, DQ], F16, tag="wv", name="wv")
            wo_sb = const.tile([128, 2, EMB], F16, tag="wo", name="wo")  # deferred DMA
            bq_sb = const.tile([128, 2], F32, tag="bq", name="bq")
            bk_sb = const.tile([128, 2], F32, tag="bk", name="bk")

            # ---- persistent activations ----
            x_all = persist.tile([128, KC, NJ, NQ], F16, tag="x", name="x")
            qT = [persist.tile([128, N], F16, tag=f"qT{p}", name=f"qT{p}") for p in range(2)]
            kT = [persist.tile([128, N], F16, tag=f"kT{p}", name=f"kT{p}") for p in range(2)]
            ctxT = [persist.tile([128, N], F16, tag=f"ctxT{p}", name=f"ctxT{p}") for p in range(2)]
            # V for 4 local heads: [nk-part, t, head*65 + (0:64 | ones)]
            v_all = persist.tile([128, NT, 4 * (HD + 1)], F16, tag="v", name="v")
            for h in range(4):
                nc.vector.memset(v_all[:, :, h * 65 + 64], 1.0)

            def x_dma(n):
                # x chunks ride the (startup-idle) GpSimd/Vector DMA queues,
                # k-granular so K/V/Q groups start on partial arrivals;
                # weights stream in parallel on the Sync queue.
                for k in range(KC):
                    eng = nc.gpsimd if k % 2 == 0 else nc.sync
                    eng.dma_start(
                        out=x_all[:, k, n, :],
                        in_=xT[k * 128:(k + 1) * 128, n * NQ:(n + 1) * NQ])

            # ---- projection parcels ----
            # kq_group(p, n): 8 accumulating MMs + bias-add, emitted as
            # 4x(2 MMs) + 1 DVE parcel so each slots into per-t ACT slack.
            _pp = {}

            def kq_mms(p, n, wsb, key, ks):
                if ks == 0:
                    _pp[key] = ppool.tile([128, NQ], F32, tag="pp", name="pp")
                ps = _pp[key]
                for k in (ks, ks + 1):
                    nc.tensor.matmul(
                        ps, wsb[:, k, p * 128:(p + 1) * 128],
                        x_all[:, k, n, :], start=(k == 0), stop=(k == KC - 1))

            def kq_fin(p, n, bsb, dst, key):
                ps = _pp.pop(key)
                nc.vector.tensor_tensor(
                    out=dst[p][:, n * NQ:(n + 1) * NQ], in0=ps,
                    in1=bsb[:, p:p + 1].broadcast_to([128, NQ]), op=add)

            def kq_parcels(p, n, wsb, bsb, dst, key):
                out = [lambda ks=ks: kq_mms(p, n, wsb, key, ks)
                       for ks in (0, 2, 4, 6)]
                out.append(lambda: kq_fin(p, n, bsb, dst, key))
                return out

            def v_mms(n, tl, key, ks):
                if ks == 0:
                    _pp[key] = ppool.tile([128, NQ], F32, tag="pp", name="pp")
                ps = _pp[key]
                for k in (ks, ks + 1):
                    nc.tensor.matmul(
                        ps[:, 0:256], x_all[:, k, n, tl * 128:(tl + 1) * 128],
                        wv_sb[:, k, :], start=(k == 0), stop=(k == KC - 1))

            def v_fin(n, tl, key):
                ps = _pp.pop(key)
                t = n * 4 + tl
                vv = v_all[:, t, :].rearrange("p (h c) -> p h c", c=65)
                nc.vector.tensor_copy(
                    out=vv[:, :, 0:64],
                    in_=ps[:, 0:256].rearrange("p (h c) -> p h c", c=64))

            def v_parcels(n, tl):
                key = ("v", n, tl)
                out = [lambda ks=ks: v_mms(n, tl, key, ks)
                       for ks in (0, 2, 4, 6)]
                out.append(lambda: v_fin(n, tl, key))
                return out

            # ---- out-projection parcels ----
            o_tiles = {}

            def out_proj_parcel(kp, m):
                # both eo halves in one parcel: the stationary ctxT chunk is
                # loaded once and serves two 512-wide matmuls.
                if kp == 0:
                    o_tiles[m] = opool.tile([128, EMB], F32, tag="o", name="o")
                o = o_tiles[m]
                pos = []
                for eo in range(2):
                    po = ppool.tile([128, NQ], F32, tag="pp", name="pp")
                    nc.tensor.matmul(
                        po, ctxT[kp][:, m * 128:(m + 1) * 128],
                        wo_sb[:, kp, eo * NQ:(eo + 1) * NQ],
                        start=True, stop=True)
                    pos.append(po)
                for eo in range(2):
                    if kp == 0:
                        nc.vector.tensor_copy(o[:, eo * NQ:(eo + 1) * NQ],
                                              pos[eo])
                    else:
                        o16 = o_tiles[("f", m)]
                        nc.vector.tensor_tensor(
                            out=o16[:, eo * NQ:(eo + 1) * NQ],
                            in0=o[:, eo * NQ:(eo + 1) * NQ], in1=pos[eo],
                            op=add)

            def out_proj_p1(m):
                o_tiles[("f", m)] = o16pool.tile([128, EMB], F16, tag="o16",
                                                 name="o16")
                out_proj_parcel(1, m)
                o16 = o_tiles.pop(("f", m))
                o_tiles.pop(m)
                nc.sync.dma_start(
                    out=out_part[m * 128:(m + 1) * 128, :], in_=o16)

            # ---- attention machinery ----
            def s_pair(p, j, t):
                sg = spool.tile([128, 2, NQ], F32, tag="s", name="s")
                for h in range(2):
                    nc.tensor.matmul(
                        sg[:, h, :],
                        kT[p][64 * h:64 * h + 64, t * 128:(t + 1) * 128],
                        qT[p][64 * h:64 * h + 64, j * NQ:(j + 1) * NQ],
                        start=True, stop=True)
                return sg

            def exp_act(sg):
                e = epool.tile([128, 2, NQ], F16, tag="e", name="e")
                nc.scalar.activation(e, sg, FP.Exp, scale=SCALE)
                return e

            def ctx_pair(p, cps, e, t):
                for h in range(2):
                    hl = 2 * p + h
                    nc.tensor.matmul(
                        cps[h], v_all[:, t, hl * 65:(hl + 1) * 65],
                        e[:, h, :], start=(t == 0), stop=(t == NT - 1))

            def normalize(p, j, h, cps):
                # copy out of PSUM first (frees the ctx bank for next j);
                # rowsum sits in row 64 -> stage to partition 0 for the
                # gpsimd broadcast, reciprocal on the [1, 512] staged row.
                cs = npool.tile([65, NQ], F32, tag="cs", name="cs")
                nc.vector.tensor_copy(cs, cps[h])
                rs = npool.tile([1, NQ], F32, tag="rs", name="rs")
                nc.vector.tensor_copy(rs, cs[64:65, :])
                rc = npool.tile([1, NQ], F32, tag="rc", name="rc")
                nc.vector.reciprocal_approx_fast(rc, rs)
                rb = npool.tile([64, NQ], F32, tag="rb", name="rb")
                nc.gpsimd.partition_broadcast(rb, rc)
                nc.vector.scalar_tensor_tensor(
                    out=ctxT[p][h * 64:(h + 1) * 64, j * NQ:(j + 1) * NQ],
                    in0=cs[0:64, :], scalar=1.0, in1=rb,
                    op0=mult, op1=mult)

            # ---- schedule ----
            # One flat software pipeline over all 128 (p, j, t) iterations:
            # iteration g emits S-pair(g), exp(g), then ctx-pair(g-2) (lag 2
            # so ctx never waits on the exp semaphore), then filler parcels
            # from sched[g]. j/p boundaries are crossed seamlessly; the
            # normalize for j is emitted right after its last ctx pair.
            sched = {}

            def put(g, f):
                sched.setdefault(g, []).append(f)

            cps_by = {}
            pend = []

            def emit_ctx(g2):
                p2, r2 = divmod(g2, 64)
                j2, t2 = divmod(r2, 16)
                if t2 == 0:
                    cps_by[(p2, j2)] = [
                        cpool.tile([65, NQ], F32, tag=f"c{h}", name=f"c{h}")
                        for h in range(2)]
                e2 = pend.pop(0)
                ctx_pair(p2, cps_by[(p2, j2)], e2, t2)
                if t2 == NT - 1:
                    cps2 = cps_by.pop((p2, j2))
                    for h in range(2):
                        normalize(p2, j2, h, cps2)

            # ---------- prefix ----------
            # Sync queue: monolithic weight DMAs; GpSimd/Vector queues: x.
            nc.sync.dma_start(out=wk_sb, in_=wkT.rearrange(
                "(k p) d -> p k d", p=128))
            nc.sync.dma_start(out=wq_sb, in_=wqT.rearrange(
                "(k p) d -> p k d", p=128))
            nc.sync.dma_start(out=wv_sb, in_=wvT.rearrange(
                "(k p) d -> p k d", p=128))
            nc.sync.dma_start(out=bk_sb, in_=bkd.rearrange("(m p) -> p m", p=128))
            nc.sync.dma_start(out=bq_sb, in_=bqd.rearrange("(m p) -> p m", p=128))
            for n in range(NJ):
                x_dma(n)
            for f in kq_parcels(0, 0, wk_sb, bk_sb, kT, ("k0", 0)):
                f()
            for f in kq_parcels(0, 0, wq_sb, bq_sb, qT, ("q0", 0)):
                f()

            # ---------- filler schedule ----------
            # pair-0 window (g 0..63):
            #   V(n, tl) whole-group at g=4n+tl+1 (ctx(t) runs at g=t+2)
            #   K0(n) parcels at g=4n-4..4n-1; Q0(n) parcels at g=16n-5..
            for n in range(NJ):
                for tl in range(4):
                    for f in v_parcels(n, tl):
                        put(4 * n + tl + 1, f)
            for n in (1, 2, 3):
                for i, f in enumerate(kq_parcels(0, n, wk_sb, bk_sb, kT,
                                                 ("k0", n))):
                    put(4 * n - 4 + min(i, 3), f)
                for i, f in enumerate(kq_parcels(0, n, wq_sb, bq_sb, qT,
                                                 ("q0", n))):
                    put(16 * n - 5 + i, f)
            put(13, lambda: nc.sync.dma_start(
                out=wo_sb, in_=woT.rearrange("(k p) e -> p k e", p=128)))
            free_iters = [g for g in range(17, 64)
                          if len(sched.get(g, [])) == 0]
            fill = []
            for n in range(NJ):
                fill += kq_parcels(1, n, wk_sb, bk_sb, kT, ("k1", n))
                fill += kq_parcels(1, n, wq_sb, bq_sb, qT, ("q1", n))
            for i, f in enumerate(fill):
                put(free_iters[i * len(free_iters) // len(fill)], f)
            # pair-1 window (g 64..127): out-proj pass 0 at g=64..95;
            # pass 1 for j at g=64+16(j+1)+2.. (8 parcels); j=3 in the tail.
            for m in range(NT):
                put(64 + 2 * m, lambda m=m: out_proj_parcel(0, m))
            for j in range(NJ - 1):
                for i, m in enumerate(range(4 * j, 4 * j + 4)):
                    put(64 + 16 * (j + 1) + 2 + 3 * i,
                        lambda m=m: out_proj_p1(m))

            # ---------- main pipeline (batches of 2 t-iterations) ----------
            # [S-pair(g), S-pair(g+1)] then [ctx(g-2), ctx(g-1)] so the PE
            # switches between 64-row-tile mode and full-array mode once per
            # batch instead of twice per iteration (each switch exposes the
            # ~150ns pipe drain to the next matmul).
            for gb in range(0, 128, 2):
                sgs = []
                for g in (gb, gb + 1):
                    p, r = divmod(g, 64)
                    j, t = divmod(r, 16)
                    sgs.append(s_pair(p, j, t))
                for sg in sgs:
                    pend.append(exp_act(sg))
                for g in (gb, gb + 1):
                    for f in sched.get(g, ()):
                        f()
                if gb >= 2:
                    emit_ctx(gb - 2)
                    emit_ctx(gb - 1)

            # ---------- tail: ctx/normalize for (p1, j3, t14..15) ----------
            # then pipelined per-m normalize-chunk + out-proj + DMA.
            p3, j3 = 1, NJ - 1
            cps3 = None
            for g2 in (126, 127):
                t2 = g2 % 16
                e2 = pend.pop(0)
                ctx_pair(p3, cps_by[(p3, j3)], e2, t2)
            cps3 = cps_by.pop((p3, j3))
            css, rbs = [], []
            for h in range(2):
                cs = npool.tile([65, NQ], F32, tag="cs", name="cs")
                nc.vector.tensor_copy(cs, cps3[h])
                rs = npool.tile([1, NQ], F32, tag="rs", name="rs")
                nc.vector.tensor_copy(rs, cs[64:65, :])
                rc = npool.tile([1, NQ], F32, tag="rc", name="rc")
                nc.vector.reciprocal_approx_fast(rc, rs)
                rb = npool.tile([64, NQ], F32, tag="rb", name="rb")
                nc.gpsimd.partition_broadcast(rb, rc)
                css.append(cs)
                rbs.append(rb)
            for m in range(4 * j3, 4 * j3 + 4):
                mo = (m - 4 * j3) * 128
                for h in range(2):
                    nc.vector.scalar_tensor_tensor(
                        out=ctxT[p3][h * 64:(h + 1) * 64,
                                     j3 * NQ + mo:j3 * NQ + mo + 128],
                        in0=css[h][0:64, mo:mo + 128], scalar=1.0,
                        in1=rbs[h][:, mo:mo + 128], op0=mult, op1=mult)
                out_proj_p1(m)

    nc.compile()
    return nc


_NC_CACHE = {}


def _get_program():
    if "nc" not in _NC_CACHE:
        _NC_CACHE["nc"] = build_program()
    return _NC_CACHE["nc"]


def make_in_maps(x, Wq, bq, Wk, bk, Wv, bv, Wo, bo):
    x = np.asarray(x)
    xTs = [np.ascontiguousarray(x[b].T.astype(np.float16)) for b in range(B)]
    in_maps = []
    for c in range(N_CORES):
        b, hg = divmod(c, TP)
        sl = slice(hg * DQ, (hg + 1) * DQ)
        in_maps.append({
            "xT": xTs[b],
            "wqT": np.ascontiguousarray(np.asarray(Wq, np.float16)[sl, :].T),
            "wkT": np.ascontiguousarray(np.asarray(Wk, np.float16)[sl, :].T),
            "wvT": np.ascontiguousarray(np.asarray(Wv, np.float16)[sl, :].T),
            "woT": np.ascontiguousarray(np.asarray(Wo, np.float16)[:, sl].T),
            "bq_s": np.ascontiguousarray(np.asarray(bq, np.float32)[sl]),
            "bk_s": np.ascontiguousarray(np.asarray(bk, np.float32)[sl]),
        })
    return in_maps


def assemble_output(results, Wv_bias_term):
    out = np.empty((B, N, EMB), np.float32)
    for b in range(B):
        acc = results[b * TP]["out_part"].astype(np.float32)
        for g in range(1, TP):
            acc = acc + results[b * TP + g]["out_part"].astype(np.float32)
        out[b] = acc + Wv_bias_term
    return out


def kernel(x, Wq, bq, Wk, bk, Wv, bv, Wo, bo):
    nc = _get_program()
    in_maps = make_in_maps(x, Wq, bq, Wk, bk, Wv, bv, Wo, bo)
    res = run_bass_kernel_spmd(nc, in_maps, list(range(N_CORES)))
    bias_term = (np.asarray(bv, np.float32) @ np.asarray(Wo, np.float32).T
                 + np.asarray(bo, np.float32))
    return assemble_output(res.results, bias_term)
